# revision 23
# baseline (speedup 1.0000x reference)
# Multi-head causal attention for 8 Trainium2 NeuronCores (Bass/Tile).
#
# Problem: q,k,v [2,16,2048,64] f32, bool mask [1,1,2048,2048] (True = masked,
# additive -1e4 bias before softmax in the reference).
#
# Sharding: batch*heads = 32 items, 4 per core (pure data/head parallel, no
# communication).
#
# Per-core kernel (per head), all in "transposed score" layout so softmax'd
# probabilities come out of the ScalarEngine already laid out for the PV
# matmul (keys on partitions):
#   - Q,K loaded naturally, transposed on-device via TensorE (paired 128x128
#     transposes) into qt/kt [64, 2048] (head-dim on partitions).
#   - Per key-block j: S^T_j = K_j Q^T via matmul (f32r, 1 cyc/row) into
#     PSUM [128, <=1024]; exp on ScalarE with the 1/sqrt(64) scale folded in
#     (no row-max subtraction: |scores| <= ~7, exp is safe in f32, and
#     softmax is shift-invariant so the result matches the reference).
#   - Mask handling, decided on the host per 128x128 block from the actual
#     mask input: fully-masked blocks are skipped outright (their probs
#     underflow to exactly 0 in the reference too); mixed blocks multiply
#     the probabilities by a 0/1 keep-tile (equivalent to the -1e4 bias:
#     exp(s - 1e4) == 0 exactly in f32) on the otherwise idle GpSimd engine.
#   - PV accumulates O^T [64, q] in PSUM over key-blocks, with V augmented
#     by a ones-column so row 64 of the accumulator is the softmax
#     denominator for free.
#   - Epilogue: copy to SBUF, transpose O^T back with TensorE, multiply by
#     the reciprocal denominator (gathered to [128,16] via a tiny SBUF->SBUF
#     DMA), DMA out.
import numpy as np
from contextlib import ExitStack

B, H, S, D = 2, 16, 2048, 64
NCORES = 8
BH = B * H
HPC = BH // NCORES  # heads per core
BLK = 128
NB = S // BLK  # 16
VW = D + 1  # V columns + ones column
SCALE = 1.0 / 8.0  # 1/sqrt(D)

FREE, SKIP, BIAS = 0, 1, 2

_cache = {}


def _plan_from_mask(mask):
    """Classify 128x128 mask blocks; build unique 0/1 keep-tiles ([key, query]
    orientation) for the mixed blocks."""
    mask2d = np.asarray(mask).reshape(S, S).astype(bool)
    m = mask2d.reshape(NB, BLK, NB, BLK)
    anyb = m.any(axis=(1, 3))
    allb = m.all(axis=(1, 3))
    codes = np.where(allb, SKIP, np.where(anyb, BIAS, FREE)).astype(np.int64)
    # A query row whose whole key range is masked sees a constant bias, which
    # softmax ignores -- the reference then equals unmasked attention. Treat
    # whole such q-blocks as unmasked.
    fq = mask2d.all(axis=1).reshape(NB, BLK).all(axis=1)
    codes[fq, :] = FREE
    tiles = {}
    tile_idx = np.full((NB, NB), -1, dtype=np.int64)
    for qb in range(NB):
        for kb in range(NB):
            if codes[qb, kb] != BIAS:
                continue
            t = np.ascontiguousarray(
                (~mask2d[qb * BLK:(qb + 1) * BLK, kb * BLK:(kb + 1) * BLK].T)
            ).astype(np.float32)
            key = t.tobytes()
            if key not in tiles:
                tiles[key] = (len(tiles), t)
            tile_idx[qb, kb] = tiles[key][0]
    if tiles:
        bt = np.stack([t for _, t in sorted(tiles.values())], axis=0)
    else:
        bt = np.zeros((1, BLK, BLK), np.float32)
    return codes, tile_idx, bt


def _ceil_pieces(c0, c1, step):
    out = []
    c = c0
    while c < c1:
        out.append((c, min(c + step, c1)))
        c = out[-1][1]
    return out


def _aligned_pieces(c0, c1, step):
    """Pieces of (c0, c1) cut at multiples of `step` (PSUM bank boundaries)."""
    out = []
    c = c0
    while c < c1:
        nxt = min((c // step + 1) * step, c1)
        out.append((c, nxt))
        c = nxt
    return out


def _runs(blocks):
    """Contiguous runs from a sorted list of block indices."""
    runs = []
    for i in blocks:
        if runs and runs[-1][1] == i:
            runs[-1][1] = i + 1
        else:
            runs.append([i, i + 1])
    return [tuple(r) for r in runs]


def build_nc(codes, tile_idx, n_bt, mmdt_name="float32r"):
    import concourse.bass as bass
    import concourse.mybir as mybir
    import concourse.tile as tile
    from concourse import bacc
    from concourse.masks import make_identity

    dt = mybir.dt
    f32 = dt.float32
    mmdt = getattr(dt, mmdt_name)
    two_byte = mmdt in (dt.float16, dt.bfloat16)
    Exp = mybir.ActivationFunctionType.Exp
    mult = mybir.AluOpType.mult

    # Per key-block: which q-blocks participate.
    active = {j: [i for i in range(NB) if codes[i, j] != SKIP] for j in range(NB)}
    for i in range(NB):
        assert any(codes[i, j] != SKIP for j in range(NB)), (
            "query block with all key blocks masked should be impossible"
        )
    # PV PSUM accumulation start/stop must be managed per 512-column PSUM
    # bank (4 q-blocks): first/last key-block writing each bank.
    NBANK = 4
    bank_first = {}
    bank_last = {}
    for bank in range(NBANK):
        js = [
            j
            for j in range(NB)
            if any(codes[i, j] != SKIP for i in range(bank * 4, bank * 4 + 4))
        ]
        bank_first[bank] = js[0]
        bank_last[bank] = js[-1]

    nc = bacc.Bacc("TRN2", target_bir_lowering=False, debug=False, num_devices=NCORES)
    q_d = nc.dram_tensor("q", [HPC, S, D], f32, kind="ExternalInput").ap()
    k_d = nc.dram_tensor("k", [HPC, S, D], f32, kind="ExternalInput").ap()
    v_d = nc.dram_tensor("v", [HPC, S, D], f32, kind="ExternalInput").ap()
    bt_d = nc.dram_tensor("bt", [n_bt, BLK, BLK], f32, kind="ExternalInput").ap()
    o_d = nc.dram_tensor("o", [HPC, S, D], f32, kind="ExternalOutput").ap()

    with tile.TileContext(nc) as tc, ExitStack() as ctx:
        const = ctx.enter_context(tc.tile_pool(name="const", bufs=1))
        ldpool = ctx.enter_context(tc.tile_pool(name="ld", bufs=3))
        tpool = ctx.enter_context(tc.tile_pool(name="tp", bufs=2))
        ppool = ctx.enter_context(tc.tile_pool(name="pp", bufs=3))
        otpool = ctx.enter_context(tc.tile_pool(name="ot", bufs=2))
        smpool = ctx.enter_context(tc.tile_pool(name="sm", bufs=2))
        outpool = ctx.enter_context(tc.tile_pool(name="ob", bufs=3))
        trspool = ctx.enter_context(tc.tile_pool(name="trs", bufs=3))
        scpool = ctx.enter_context(tc.tile_pool(name="sc", bufs=2, space="PSUM"))
        pvpool = ctx.enter_context(tc.tile_pool(name="pv", bufs=2, space="PSUM"))

        ident = const.tile([BLK, BLK], f32, tag="ident")
        make_identity(nc, ident[:])
        if two_byte:
            # separate identity in the matmul dtype for the q/k transposes
            identm = const.tile([BLK, BLK], mmdt, tag="identm")
            make_identity(nc, identm[:])
        else:
            identm = ident
        bts = []
        for u in range(n_bt):
            t = const.tile(
                [BLK, BLK], mmdt if two_byte else f32, tag=f"bt{u}", name=f"bt_sb{u}"
            )
            if two_byte:
                nc.gpsimd.dma_start(out=t[:], in_=bt_d[u])  # casts f32 -> mmdt
            else:
                nc.sync.dma_start(out=t[:], in_=bt_d[u])
            bts.append(t)

        for h in range(HPC):
            # ---- load this head's Q, K, V (blocked natural layout) ----
            # With a 2-byte matmul dtype, the f32->mmdt cast happens inside
            # the (SWDGE) DMA itself; otherwise load f32 and cast on DVE.
            ldt = mmdt if two_byte else f32
            dma_ld = nc.gpsimd.dma_start if two_byte else nc.sync.dma_start
            qn = ldpool.tile([BLK, NB * D], ldt, tag="qn")
            dma_ld(
                out=qn[:].rearrange("p (n d) -> p n d", d=D),
                in_=q_d[h].rearrange("(n p) d -> p n d", p=BLK),
            )
            kn = ldpool.tile([BLK, NB * D], ldt, tag="kn")
            dma_ld(
                out=kn[:].rearrange("p (n d) -> p n d", d=D),
                in_=k_d[h].rearrange("(n p) d -> p n d", p=BLK),
            )
            vno = ldpool.tile([BLK, NB * VW], mmdt, tag="vn")
            vno3 = vno[:].rearrange("p (n c) -> p n c", c=VW)
            if two_byte:
                nc.gpsimd.dma_start(
                    out=vno3[:, :, 0:D],
                    in_=v_d[h].rearrange("(n p) d -> p n d", p=BLK),
                )
                ones_src = vno3[:, :, 0:1]
            else:
                vld = ldpool.tile([BLK, NB * D], f32, tag="vld")
                nc.sync.dma_start(
                    out=vld[:].rearrange("p (n d) -> p n d", d=D),
                    in_=v_d[h].rearrange("(n p) d -> p n d", p=BLK),
                )
                nc.vector.tensor_copy(
                    vno3[:, :, 0:D], vld[:].rearrange("p (n d) -> p n d", d=D)
                )
                ones_src = vld[:].rearrange("p (n d) -> p n d", d=D)[:, :, 0:1]
            nc.vector.tensor_scalar(
                vno3[:, :, D:VW],
                ones_src,
                0.0,
                1.0,
                mybir.AluOpType.mult,
                mybir.AluOpType.add,
            )

            # ---- transpose Q and K into [64, S] (head dim on partitions) ----
            qt = tpool.tile([D, S], mmdt, tag="qt")
            kt = tpool.tile([D, S], mmdt, tag="kt")
            for src, dst in ((qn, qt), (kn, kt)):
                for g in range(2):  # 2 groups x 4 paired transposes
                    if two_byte:
                        tr = trspool.tile([BLK, 512], mmdt, tag="trs")
                        for u in range(4):
                            t = g * 4 + u
                            nc.sync.dma_start(
                                out=tr[:, u * BLK:(u + 1) * BLK],
                                in_=src[:, t * BLK:(t + 1) * BLK],
                                transpose=True,
                            )
                    else:
                        tr = scpool.tile([BLK, 512], ldt, tag="sc")
                        for u in range(4):
                            t = g * 4 + u
                            nc.tensor.transpose(
                                tr[:, u * BLK:(u + 1) * BLK],
                                src[:, t * BLK:(t + 1) * BLK],
                                ident[:],
                            )
                    half = dst[:, g * 1024:(g + 1) * 1024].rearrange(
                        "p (u c) -> p u c", c=256
                    )
                    nc.vector.tensor_copy(
                        half[:, :, 0:BLK],
                        tr[0:D, :].rearrange("p (u c) -> p u c", c=BLK),
                    )
                    nc.vector.tensor_copy(
                        half[:, :, BLK:256],
                        tr[D:BLK, :].rearrange("p (u c) -> p u c", c=BLK),
                    )

            # ---- main loop over key blocks ----
            pvh = [
                pvpool.tile([VW, 1024], f32, tag="pv", name=f"pv{h}_{i}")
                for i in range(2)
            ]
            for j in range(NB):
                blocks = active[j]
                if not blocks:
                    continue
                pT = ppool.tile([BLK, S], mmdt, tag="pT")
                for (r0, r1) in _runs(blocks):
                    for (c0, c1) in _ceil_pieces(r0 * BLK, r1 * BLK, 1024):
                        w = c1 - c0
                        sc = scpool.tile([BLK, w], f32, tag="sc")
                        for (s0, s1) in _ceil_pieces(0, w, 512):
                            nc.tensor.matmul(
                                sc[:, s0:s1],
                                lhsT=kt[:, j * BLK:(j + 1) * BLK],
                                rhs=qt[:, c0 + s0:c0 + s1],
                                start=True,
                                stop=True,
                            )
                        nc.scalar.activation(pT[:, c0:c1], sc[:, 0:w], Exp, scale=SCALE)
                # mixed blocks: zero the masked probabilities (gpsimd is idle)
                for i in blocks:
                    if codes[i, j] == BIAS:
                        sl = pT[:, i * BLK:(i + 1) * BLK]
                        nc.gpsimd.tensor_tensor(sl, sl, bts[tile_idx[i, j]][:], mult)
                # PV accumulation: start/stop flags at PSUM-bank granularity
                for bank in range(NBANK):
                    bi = [i for i in blocks if bank * 4 <= i < bank * 4 + 4]
                    if not bi:
                        continue
                    half = bank // 2
                    toff = half * 1024  # tile-relative offset of this half
                    is_last = j == bank_last[bank]
                    if j == bank_first[bank]:
                        # first write: one full-bank matmul so every column
                        # starts with start=True; zero any inactive columns
                        # of pT first (no-op for causal/empty masks).
                        for i in range(bank * 4, bank * 4 + 4):
                            if i not in bi:
                                nc.gpsimd.memset(
                                    pT[:, i * BLK:(i + 1) * BLK], 0.0
                                )
                        g0, g1 = bank * 4 * BLK, (bank + 1) * 4 * BLK
                        nc.tensor.matmul(
                            pvh[half][:, g0 - toff:g1 - toff],
                            lhsT=vno3[:, j, :],
                            rhs=pT[:, g0:g1],
                            start=True,
                            stop=is_last,
                        )
                    else:
                        runs = _runs(bi)
                        for ri, (r0, r1) in enumerate(runs):
                            nc.tensor.matmul(
                                pvh[half][:, r0 * BLK - toff:r1 * BLK - toff],
                                lhsT=vno3[:, j, :],
                                rhs=pT[:, r0 * BLK:r1 * BLK],
                                start=False,
                                stop=is_last and ri == len(runs) - 1,
                            )

            # ---- epilogue: normalize and write out ----
            # O^T and the denominators leave PSUM once ([VW, S] copy); the
            # transposes back to [q, d] run on the DMA xbar (fp16) or
            # TensorE (fallback), then one fused multiply per 8 q-blocks.
            odt = mmdt if two_byte else f32
            ot = otpool.tile([VW, S], odt, tag="ot")
            for half in range(2):
                nc.vector.tensor_copy(
                    ot[:, half * 1024:(half + 1) * 1024], pvh[half][:, :]
                )
            # gather denominators [1, S] -> [16, 128] (DMA) -> [128, 16]
            dq = smpool.tile([NB, BLK], odt, tag="dq")
            nc.sync.dma_start(out=dq[:], in_=ot[D:VW, :])
            rcp = smpool.tile([BLK, NB], f32, tag="rcp")
            if two_byte:
                dnt = smpool.tile([BLK, NB], odt, tag="dnt")
                nc.sync.dma_start(out=dnt[:], in_=dq[:], transpose=True)
                nc.vector.reciprocal(rcp[:], dnt[:])
            else:
                dntp = pvpool.tile([BLK, NB], f32, tag="pv", name=f"dnt{h}")
                nc.tensor.transpose(dntp[:], dq[:], ident[0:NB, 0:NB])
                nc.vector.reciprocal(rcp[:], dntp[:])
            for half in range(2):
                if two_byte:
                    rt = trspool.tile([BLK, 512], odt, tag="rts")
                    for u in range(8):
                        i = half * 8 + u
                        nc.sync.dma_start(
                            out=rt[:, u * D:(u + 1) * D],
                            in_=ot[0:D, i * BLK:(i + 1) * BLK],
                            transpose=True,
                        )
                else:
                    rt = pvpool.tile([BLK, 512], f32, tag="pv", name=f"rt{h}_{half}")
                    for u in range(8):
                        i = half * 8 + u
                        nc.tensor.transpose(
                            rt[:, u * D:(u + 1) * D],
                            ot[0:D, i * BLK:(i + 1) * BLK],
                            ident[0:D, 0:D],
                        )
                osb = outpool.tile([BLK, 512], f32, tag="ob")
                nc.vector.tensor_tensor(
                    osb[:].rearrange("p (u d) -> p u d", d=D),
                    rt[:].rearrange("p (u d) -> p u d", d=D),
                    rcp[:, half * 8:(half + 1) * 8]
                    .rearrange("p (u o) -> p u o", o=1)
                    .broadcast_to([BLK, 8, D]),
                    mult,
                )
                nc.sync.dma_start(
                    out=o_d[h].rearrange("(n p) d -> p n d", p=BLK)[
                        :, half * 8:(half + 1) * 8, :
                    ],
                    in_=osb[:].rearrange("p (u d) -> p u d", d=D),
                )
    nc.compile()
    return nc


MM_DT = __import__("os").environ.get("ATTN_MM_DT", "float16")


def _get_program(mask):
    codes, tile_idx, bt = _plan_from_mask(mask)
    key = (codes.tobytes(), tile_idx.tobytes(), bt.tobytes(), MM_DT)
    if key not in _cache:
        _cache[key] = (build_nc(codes, tile_idx, bt.shape[0], MM_DT), bt)
    return _cache[key]


LAST_RESULTS = None  # BassKernelResults of the most recent run (for profiling)


def kernel(q, k, v, mask):
    global LAST_RESULTS
    from concourse.bass_utils import run_bass_kernel_spmd

    nc, bt = _get_program(mask)
    qf = np.asarray(q, np.float32).reshape(BH, S, D)
    kf = np.asarray(k, np.float32).reshape(BH, S, D)
    vf = np.asarray(v, np.float32).reshape(BH, S, D)
    in_maps = [
        {
            "q": qf[c * HPC:(c + 1) * HPC],
            "k": kf[c * HPC:(c + 1) * HPC],
            "v": vf[c * HPC:(c + 1) * HPC],
            "bt": bt,
        }
        for c in range(NCORES)
    ]
    res = run_bass_kernel_spmd(nc, in_maps, list(range(NCORES)))
    LAST_RESULTS = res
    out = np.concatenate([res.results[c]["o"] for c in range(NCORES)], axis=0)
    return out.reshape(B, H, S, D).astype(np.float32)


# revision 26
# speedup vs baseline: 1.6250x; 1.6250x over previous
# Multi-head causal attention for 8 Trainium2 NeuronCores (Bass/Tile).
#
# Problem: q,k,v [2,16,2048,64] f32, bool mask [1,1,2048,2048] (True = masked,
# additive -1e4 bias before softmax in the reference).
#
# Sharding: batch*heads = 32 items, 4 per core (pure data/head parallel, no
# communication).
#
# Per-core kernel (per head), all in "transposed score" layout so softmax'd
# probabilities come out of the ScalarEngine already laid out for the PV
# matmul (keys on partitions):
#   - Q,K loaded naturally, transposed on-device via TensorE (paired 128x128
#     transposes) into qt/kt [64, 2048] (head-dim on partitions).
#   - Per key-block j: S^T_j = K_j Q^T via matmul (f32r, 1 cyc/row) into
#     PSUM [128, <=1024]; exp on ScalarE with the 1/sqrt(64) scale folded in
#     (no row-max subtraction: |scores| <= ~7, exp is safe in f32, and
#     softmax is shift-invariant so the result matches the reference).
#   - Mask handling, decided on the host per 128x128 block from the actual
#     mask input: fully-masked blocks are skipped outright (their probs
#     underflow to exactly 0 in the reference too); mixed blocks multiply
#     the probabilities by a 0/1 keep-tile (equivalent to the -1e4 bias:
#     exp(s - 1e4) == 0 exactly in f32) on the otherwise idle GpSimd engine.
#   - PV accumulates O^T [64, q] in PSUM over key-blocks, with V augmented
#     by a ones-column so row 64 of the accumulator is the softmax
#     denominator for free.
#   - Epilogue: copy to SBUF, transpose O^T back with TensorE, multiply by
#     the reciprocal denominator (gathered to [128,16] via a tiny SBUF->SBUF
#     DMA), DMA out.
import numpy as np
from contextlib import ExitStack

B, H, S, D = 2, 16, 2048, 64
NCORES = 8
BH = B * H
HPC = BH // NCORES  # heads per core
BLK = 128
NB = S // BLK  # 16
VW = D + 1  # V columns + ones column
SCALE = 1.0 / 8.0  # 1/sqrt(D)

FREE, SKIP, BIAS = 0, 1, 2

_cache = {}


def _plan_from_mask(mask):
    """Classify 128x128 mask blocks; build unique 0/1 keep-tiles ([key, query]
    orientation) for the mixed blocks."""
    mask2d = np.asarray(mask).reshape(S, S).astype(bool)
    m = mask2d.reshape(NB, BLK, NB, BLK)
    anyb = m.any(axis=(1, 3))
    allb = m.all(axis=(1, 3))
    codes = np.where(allb, SKIP, np.where(anyb, BIAS, FREE)).astype(np.int64)
    # A query row whose whole key range is masked sees a constant bias, which
    # softmax ignores -- the reference then equals unmasked attention. Treat
    # whole such q-blocks as unmasked.
    fq = mask2d.all(axis=1).reshape(NB, BLK).all(axis=1)
    codes[fq, :] = FREE
    tiles = {}
    tile_idx = np.full((NB, NB), -1, dtype=np.int64)
    for qb in range(NB):
        for kb in range(NB):
            if codes[qb, kb] != BIAS:
                continue
            t = np.ascontiguousarray(
                (~mask2d[qb * BLK:(qb + 1) * BLK, kb * BLK:(kb + 1) * BLK].T)
            ).astype(np.float32)
            key = t.tobytes()
            if key not in tiles:
                tiles[key] = (len(tiles), t)
            tile_idx[qb, kb] = tiles[key][0]
    if tiles:
        bt = np.stack([t for _, t in sorted(tiles.values())], axis=0)
    else:
        bt = np.zeros((1, BLK, BLK), np.float32)
    return codes, tile_idx, bt


def _ceil_pieces(c0, c1, step):
    out = []
    c = c0
    while c < c1:
        out.append((c, min(c + step, c1)))
        c = out[-1][1]
    return out


def _aligned_pieces(c0, c1, step):
    """Pieces of (c0, c1) cut at multiples of `step` (PSUM bank boundaries)."""
    out = []
    c = c0
    while c < c1:
        nxt = min((c // step + 1) * step, c1)
        out.append((c, nxt))
        c = nxt
    return out


def _runs(blocks):
    """Contiguous runs from a sorted list of block indices."""
    runs = []
    for i in blocks:
        if runs and runs[-1][1] == i:
            runs[-1][1] = i + 1
        else:
            runs.append([i, i + 1])
    return [tuple(r) for r in runs]


def build_nc(codes, tile_idx, n_bt, mmdt_name="float32r"):
    import concourse.bass as bass
    import concourse.mybir as mybir
    import concourse.tile as tile
    from concourse import bacc
    from concourse.masks import make_identity
    from concourse.tile_rust import add_dep_helper

    dt = mybir.dt
    f32 = dt.float32
    mmdt = getattr(dt, mmdt_name)
    two_byte = mmdt in (dt.float16, dt.bfloat16)
    use_xbar = False  # DMA-xbar transposes measured slower (serialized ~1.2us each)
    Exp = mybir.ActivationFunctionType.Exp
    mult = mybir.AluOpType.mult

    # Per key-block: which q-blocks participate.
    active = {j: [i for i in range(NB) if codes[i, j] != SKIP] for j in range(NB)}
    for i in range(NB):
        assert any(codes[i, j] != SKIP for j in range(NB)), (
            "query block with all key blocks masked should be impossible"
        )
    # PV PSUM accumulation start/stop must be managed per 512-column PSUM
    # bank (4 q-blocks): first/last key-block writing each bank.
    NBANK = 4
    bank_first = {}
    bank_last = {}
    for bank in range(NBANK):
        js = [
            j
            for j in range(NB)
            if any(codes[i, j] != SKIP for i in range(bank * 4, bank * 4 + 4))
        ]
        bank_first[bank] = js[0]
        bank_last[bank] = js[-1]

    nc = bacc.Bacc("TRN2", target_bir_lowering=False, debug=False, num_devices=NCORES)
    q_d = nc.dram_tensor("q", [HPC, S, D], f32, kind="ExternalInput").ap()
    k_d = nc.dram_tensor("k", [HPC, S, D], f32, kind="ExternalInput").ap()
    v_d = nc.dram_tensor("v", [HPC, S, D], f32, kind="ExternalInput").ap()
    bt_d = nc.dram_tensor("bt", [n_bt, BLK, BLK], f32, kind="ExternalInput").ap()
    o_d = nc.dram_tensor("o", [HPC, S, D], f32, kind="ExternalOutput").ap()

    with tile.TileContext(nc) as tc, ExitStack() as ctx:
        const = ctx.enter_context(tc.tile_pool(name="const", bufs=1))
        ldpool = ctx.enter_context(tc.tile_pool(name="ld", bufs=3))
        tpool = ctx.enter_context(tc.tile_pool(name="tp", bufs=2))
        ppool = ctx.enter_context(tc.tile_pool(name="pp", bufs=3))
        otpool = ctx.enter_context(tc.tile_pool(name="ot", bufs=2))
        smpool = ctx.enter_context(tc.tile_pool(name="sm", bufs=2))
        outpool = ctx.enter_context(tc.tile_pool(name="ob", bufs=3))
        trspool = ctx.enter_context(tc.tile_pool(name="trs", bufs=3))
        scpool = ctx.enter_context(tc.tile_pool(name="sc", bufs=2, space="PSUM"))
        pvpool = ctx.enter_context(tc.tile_pool(name="pv", bufs=2, space="PSUM"))

        ident = const.tile([BLK, BLK], f32, tag="ident")
        make_identity(nc, ident[:])
        if two_byte:
            # separate identity in the matmul dtype for the q/k transposes
            identm = const.tile([BLK, BLK], mmdt, tag="identm")
            make_identity(nc, identm[:])
        else:
            identm = ident
        bts = []
        for u in range(n_bt):
            t = const.tile(
                [BLK, BLK], mmdt if two_byte else f32, tag=f"bt{u}", name=f"bt_sb{u}"
            )
            if two_byte:
                nc.gpsimd.dma_start(out=t[:], in_=bt_d[u])  # casts f32 -> mmdt
            else:
                nc.sync.dma_start(out=t[:], in_=bt_d[u])
            bts.append(t)

        # PE-order bookkeeping: weight reloads cost ~330ns per stationary
        # swap, so same-weight matmuls must run adjacently. We collect the
        # PE instructions and chain them (sync=False deps) in a software-
        # pipelined order: transposes, QK_j+1 before PV_j, epilogue
        # transposes of head h slotted early into head h+1's stream.
        trans_h = []
        qk_h = []
        pv_h = []
        epi_h = []
        for h in range(HPC):
            trans_insts = []
            qk_groups = []
            pv_groups = []
            epi_insts = []
            # ---- load this head's Q, K, V (blocked natural layout) ----
            # With a 2-byte matmul dtype, the f32->mmdt cast happens inside
            # the (SWDGE) DMA itself; otherwise load f32 and cast on DVE.
            ldt = mmdt if two_byte else f32
            dma_ld = nc.gpsimd.dma_start if two_byte else nc.sync.dma_start
            qn = ldpool.tile([BLK, NB * D], ldt, tag="qn")
            dma_ld(
                out=qn[:].rearrange("p (n d) -> p n d", d=D),
                in_=q_d[h].rearrange("(n p) d -> p n d", p=BLK),
            )
            kn = ldpool.tile([BLK, NB * D], ldt, tag="kn")
            dma_ld(
                out=kn[:].rearrange("p (n d) -> p n d", d=D),
                in_=k_d[h].rearrange("(n p) d -> p n d", p=BLK),
            )
            vno = ldpool.tile([BLK, NB * VW], mmdt, tag="vn")
            vno3 = vno[:].rearrange("p (n c) -> p n c", c=VW)
            if two_byte:
                nc.gpsimd.dma_start(
                    out=vno3[:, :, 0:D],
                    in_=v_d[h].rearrange("(n p) d -> p n d", p=BLK),
                )
                ones_src = vno3[:, :, 0:1]
            else:
                vld = ldpool.tile([BLK, NB * D], f32, tag="vld")
                nc.sync.dma_start(
                    out=vld[:].rearrange("p (n d) -> p n d", d=D),
                    in_=v_d[h].rearrange("(n p) d -> p n d", p=BLK),
                )
                nc.vector.tensor_copy(
                    vno3[:, :, 0:D], vld[:].rearrange("p (n d) -> p n d", d=D)
                )
                ones_src = vld[:].rearrange("p (n d) -> p n d", d=D)[:, :, 0:1]
            nc.vector.tensor_scalar(
                vno3[:, :, D:VW],
                ones_src,
                0.0,
                1.0,
                mybir.AluOpType.mult,
                mybir.AluOpType.add,
            )

            # ---- transpose Q and K into [64, S] (head dim on partitions) ----
            qt = tpool.tile([D, S], mmdt, tag="qt")
            kt = tpool.tile([D, S], mmdt, tag="kt")
            for src, dst in ((qn, qt), (kn, kt)):
                for g in range(2):  # 2 groups x 4 paired transposes
                    if use_xbar:
                        tr = trspool.tile([BLK, 512], mmdt, tag="trs")
                        for u in range(4):
                            t = g * 4 + u
                            nc.sync.dma_start(
                                out=tr[:, u * BLK:(u + 1) * BLK],
                                in_=src[:, t * BLK:(t + 1) * BLK],
                                transpose=True,
                            )
                    else:
                        tr = scpool.tile([BLK, 512], ldt, tag="sc")
                        for u in range(4):
                            t = g * 4 + u
                            trans_insts.append(nc.tensor.transpose(
                                tr[:, u * BLK:(u + 1) * BLK],
                                src[:, t * BLK:(t + 1) * BLK],
                                identm[:] if two_byte else ident[:],
                            ))
                    half = dst[:, g * 1024:(g + 1) * 1024].rearrange(
                        "p (u c) -> p u c", c=256
                    )
                    nc.vector.tensor_copy(
                        half[:, :, 0:BLK],
                        tr[0:D, :].rearrange("p (u c) -> p u c", c=BLK),
                    )
                    nc.vector.tensor_copy(
                        half[:, :, BLK:256],
                        tr[D:BLK, :].rearrange("p (u c) -> p u c", c=BLK),
                    )

            # ---- main loop over key blocks ----
            pvh = [
                pvpool.tile([VW, 1024], f32, tag="pv", name=f"pv{h}_{i}")
                for i in range(2)
            ]
            for j in range(NB):
                blocks = active[j]
                if not blocks:
                    continue
                pT = ppool.tile([BLK, S], mmdt, tag="pT")
                qk_g = []
                for (r0, r1) in _runs(blocks):
                    for (c0, c1) in _ceil_pieces(r0 * BLK, r1 * BLK, 1024):
                        w = c1 - c0
                        sc = scpool.tile([BLK, w], f32, tag="sc")
                        for (s0, s1) in _ceil_pieces(0, w, 512):
                            qk_g.append(nc.tensor.matmul(
                                sc[:, s0:s1],
                                lhsT=kt[:, j * BLK:(j + 1) * BLK],
                                rhs=qt[:, c0 + s0:c0 + s1],
                                start=True,
                                stop=True,
                            ))
                        nc.scalar.activation(pT[:, c0:c1], sc[:, 0:w], Exp, scale=SCALE)
                qk_groups.append(qk_g)
                pv_g = []
                # mixed blocks: zero the masked probabilities (gpsimd is idle)
                for i in blocks:
                    if codes[i, j] == BIAS:
                        sl = pT[:, i * BLK:(i + 1) * BLK]
                        nc.gpsimd.tensor_tensor(sl, sl, bts[tile_idx[i, j]][:], mult)
                # PV accumulation: start/stop flags at PSUM-bank granularity
                for bank in range(NBANK):
                    bi = [i for i in blocks if bank * 4 <= i < bank * 4 + 4]
                    if not bi:
                        continue
                    half = bank // 2
                    toff = half * 1024  # tile-relative offset of this half
                    is_last = j == bank_last[bank]
                    if j == bank_first[bank]:
                        # first write: one full-bank matmul so every column
                        # starts with start=True; zero any inactive columns
                        # of pT first (no-op for causal/empty masks).
                        for i in range(bank * 4, bank * 4 + 4):
                            if i not in bi:
                                nc.gpsimd.memset(
                                    pT[:, i * BLK:(i + 1) * BLK], 0.0
                                )
                        g0, g1 = bank * 4 * BLK, (bank + 1) * 4 * BLK
                        pv_g.append(nc.tensor.matmul(
                            pvh[half][:, g0 - toff:g1 - toff],
                            lhsT=vno3[:, j, :],
                            rhs=pT[:, g0:g1],
                            start=True,
                            stop=is_last,
                        ))
                    else:
                        runs = _runs(bi)
                        for ri, (r0, r1) in enumerate(runs):
                            pv_g.append(nc.tensor.matmul(
                                pvh[half][:, r0 * BLK - toff:r1 * BLK - toff],
                                lhsT=vno3[:, j, :],
                                rhs=pT[:, r0 * BLK:r1 * BLK],
                                start=False,
                                stop=is_last and ri == len(runs) - 1,
                            ))
                pv_groups.append(pv_g)

            # ---- epilogue: normalize and write out ----
            # O^T and the denominators leave PSUM once ([VW, S] copy); the
            # transposes back to [q, d] run on the DMA xbar (fp16) or
            # TensorE (fallback), then one fused multiply per 8 q-blocks.
            odt = mmdt if use_xbar else f32
            ot = otpool.tile([VW, S], odt, tag="ot")
            for half in range(2):
                nc.vector.tensor_copy(
                    ot[:, half * 1024:(half + 1) * 1024], pvh[half][:, :]
                )
            # gather denominators [1, S] -> [16, 128] (DMA) -> [128, 16]
            dq = smpool.tile([NB, BLK], odt, tag="dq")
            nc.sync.dma_start(out=dq[:], in_=ot[D:VW, :])
            rcp = smpool.tile([BLK, NB], f32, tag="rcp")
            if use_xbar:
                dnt = smpool.tile([BLK, NB], odt, tag="dnt")
                nc.sync.dma_start(out=dnt[:], in_=dq[:], transpose=True)
                nc.vector.reciprocal(rcp[:], dnt[:])
            else:
                dntp = pvpool.tile([BLK, NB], f32, tag="pv", name=f"dnt{h}")
                epi_insts.append(nc.tensor.transpose(dntp[:], dq[:], ident[0:NB, 0:NB]))
                nc.vector.reciprocal(rcp[:], dntp[:])
            for half in range(2):
                if use_xbar:
                    rt = trspool.tile([BLK, 512], odt, tag="rts")
                    for u in range(8):
                        i = half * 8 + u
                        nc.sync.dma_start(
                            out=rt[:, u * D:(u + 1) * D],
                            in_=ot[0:D, i * BLK:(i + 1) * BLK],
                            transpose=True,
                        )
                else:
                    rt = pvpool.tile([BLK, 512], f32, tag="pv", name=f"rt{h}_{half}")
                    for u in range(8):
                        i = half * 8 + u
                        epi_insts.append(nc.tensor.transpose(
                            rt[:, u * D:(u + 1) * D],
                            ot[0:D, i * BLK:(i + 1) * BLK],
                            ident[0:D, 0:D],
                        ))
                osb = outpool.tile([BLK, 512], f32, tag="ob")
                nc.vector.tensor_tensor(
                    osb[:].rearrange("p (u d) -> p u d", d=D),
                    rt[:].rearrange("p (u d) -> p u d", d=D),
                    rcp[:, half * 8:(half + 1) * 8]
                    .rearrange("p (u o) -> p u o", o=1)
                    .broadcast_to([BLK, 8, D]),
                    mult,
                )
                nc.sync.dma_start(
                    out=o_d[h].rearrange("(n p) d -> p n d", p=BLK)[
                        :, half * 8:(half + 1) * 8, :
                    ],
                    in_=osb[:].rearrange("p (u d) -> p u d", d=D),
                )
            trans_h.append(trans_insts)
            qk_h.append(qk_groups)
            pv_h.append(pv_groups)
            epi_h.append(epi_insts)

        # Build the PE ordering chain.
        chain = []
        for h in range(HPC):
            chain += trans_h[h]
            qk = qk_h[h]
            pv = pv_h[h]
            assert len(qk) == len(pv)
            if qk:
                chain += qk[0]
            if h > 0:
                chain += epi_h[h - 1]  # prev head's epilogue early in this head
            for idx in range(1, len(qk)):
                chain += qk[idx]
                chain += pv[idx - 1]
            if pv:
                chain += pv[-1]
        chain += epi_h[HPC - 1]
        for a, b in zip(chain, chain[1:]):
            add_dep_helper(b.ins, a.ins, sync=False, reason="pe weight-group order")
    nc.compile()
    return nc


MM_DT = __import__("os").environ.get("ATTN_MM_DT", "float16")


def _get_program(mask):
    codes, tile_idx, bt = _plan_from_mask(mask)
    key = (codes.tobytes(), tile_idx.tobytes(), bt.tobytes(), MM_DT)
    if key not in _cache:
        _cache[key] = (build_nc(codes, tile_idx, bt.shape[0], MM_DT), bt)
    return _cache[key]


LAST_RESULTS = None  # BassKernelResults of the most recent run (for profiling)


def kernel(q, k, v, mask):
    global LAST_RESULTS
    from concourse.bass_utils import run_bass_kernel_spmd

    nc, bt = _get_program(mask)
    qf = np.asarray(q, np.float32).reshape(BH, S, D)
    kf = np.asarray(k, np.float32).reshape(BH, S, D)
    vf = np.asarray(v, np.float32).reshape(BH, S, D)
    in_maps = [
        {
            "q": qf[c * HPC:(c + 1) * HPC],
            "k": kf[c * HPC:(c + 1) * HPC],
            "v": vf[c * HPC:(c + 1) * HPC],
            "bt": bt,
        }
        for c in range(NCORES)
    ]
    res = run_bass_kernel_spmd(nc, in_maps, list(range(NCORES)))
    LAST_RESULTS = res
    out = np.concatenate([res.results[c]["o"] for c in range(NCORES)], axis=0)
    return out.reshape(B, H, S, D).astype(np.float32)
